# revision 1
# baseline (speedup 1.0000x reference)
"""GCN + batch-attention kernel for Trainium2 (8 NeuronCores, SPMD).

Problem (nn_GCNResnet): for x [8192,3,10], A [3,3], W [10,10]:
    adj   = 0.25*(off_diag_ones + A)                    # normalized adjacency
    pooled= 0.5*(h0+h1),  h = adj @ (x @ W)             # -> [B,10]
    v     = softmax(pooled @ pooled.T) @ pooled         # -> [B,10]

pooled = x2 @ Wc where x2 = x.reshape(B,30) (padded to 32 features on the
host: col 30 = 1 bias feature, col 31 = 0) and Wc [32,12] is the host-folded
weight (cols 0:10 = 0.5*(adj[0,n]+adj[1,n]) * W[f,o]; col 10 selects the ones
feature, producing the augmented-V ones column; col 11 = 0 pad so every
fp32r matmul shape is even).

Per core i (batch-sharded attention; input rolled by 1024*i rows so the
identical SPMD program always works on local rows 0:1024):
  - x2T via PE transposes (batched: one [128,(4,32)] transpose per 512 rows)
  - pooledT [12,8192] = Wc.T @ x2T; natural-layout [pooled|1|0] rows -> vn
  - flash attention, never materializing the [B,B] score matrix:
      for kv chunk c (128 rows):  S.T = pooledT[:,c]^T @ pooledT[:,0:1024]
      E = exp(S.T) on ACT;  acc[12,1024] += vn[c]^T @ E on PE (PSUM)
      v = acc[0:10]/acc[10]
The per-supergroup prologue is interleaved with the attention chunks so the
scalar engine (exp, the throughput floor: 64 x [128,1024] activations) is
kept busy end to end. Matmuls use float32r (full-rate fp32, ~1e-4 rounding;
exact fp32 streams at 1/4 rate).
"""

import numpy as np

import concourse.bass as bass
import concourse.bacc as bacc
import concourse.mybir as mybir
import concourse.tile as tile
from concourse.bass_utils import run_bass_kernel_spmd

B = 8192
NCORES = 8
QL = B // NCORES          # 1024 local query rows
NF = 32                   # 30 feats + ones + zero pad
D = 10
DV = 12                   # [pooled | 1 | 0]
NSG = 8                   # supergroups of 1024 batch rows
NKV = B // 128            # 64 kv chunks

f32 = mybir.dt.float32
f32r = mybir.dt.float32r
EXP = mybir.ActivationFunctionType.Exp

_NC = None


def _build():
    nc = bacc.Bacc(trn_type="TRN2", target_bir_lowering=False)

    xr = nc.dram_tensor("xr", [B, NF], f32r, kind="ExternalInput")
    wc = nc.dram_tensor("wc", [NF, DV], f32r, kind="ExternalInput")
    identr = nc.dram_tensor("identr", [128, 128], f32r, kind="ExternalInput")
    ident12 = nc.dram_tensor("ident12", [DV, DV], f32, kind="ExternalInput")
    v = nc.dram_tensor("v", [QL, D], f32, kind="ExternalOutput")

    with tile.TileContext(nc) as tc:
        with (
            tc.tile_pool(name="const", bufs=1) as const,
            tc.tile_pool(name="xin", bufs=3) as xin,
            tc.tile_pool(name="x2tp", bufs=3) as x2tp,
            tc.tile_pool(name="bigp", bufs=1) as bigp,
            tc.tile_pool(name="epool", bufs=3) as epool,
            tc.tile_pool(name="outp", bufs=8) as outp,
            tc.tile_pool(name="ps", bufs=4, space="PSUM") as ps,
        ):
            wc_sb = const.tile([NF, DV], f32r, tag="wc")
            idr_sb = const.tile([128, 128], f32r, tag="idr")
            id12_sb = const.tile([DV, DV], f32, tag="id12")

            pooledT = bigp.tile([DV, B], f32r, tag="pooledT")
            vn = bigp.tile([128, NKV, DV], f32r, tag="vn")

            pv = ps.tile([DV, QL], f32, tag="ps")  # attention accumulator

            # PE warm-up with no DMA dependency (memset zeros, fp32 matmuls
            # at 4 cyc/row keep PE busy ~2us) so the HAM clock gate reaches
            # full rate before the real startup chain; plus one dummy ACT op
            # to pull the LoadActFuncSet (~1.3us) off the first-copy path.
            wz = const.tile([128, 128], f32, tag="wz")
            nc.vector.memset(wz[:, :], 0.0)
            actwarm = const.tile([2, 2], f32, tag="actwarm")
            nc.scalar.copy(actwarm[:, :], wz[0:2, 0:2])
            warm = ps.tile([128, 64], f32, tag="ps")
            for _ in range(4):
                nc.tensor.matmul(
                    warm[:, :], wz[:, :], wz[:, 0:64], start=True, stop=True,
                )

            # first supergroup's x lands before the small constants: the
            # SP queue issues DMAs in order (~500ns each) and x is on the
            # critical path to the first exp.
            xg0 = xin.tile([128, 8, NF], f32r, tag="xg")
            nc.sync.dma_start(
                out=xg0[:, 0:4, :],
                in_=bass.AP(xr, 0, [[NF, 128], [128 * NF, 4], [1, NF]]))
            nc.sync.dma_start(out=idr_sb[:, :], in_=identr[:, :])
            nc.sync.dma_start(
                out=xg0[:, 4:8, :],
                in_=bass.AP(xr, 512 * NF, [[NF, 128], [128 * NF, 4], [1, NF]]))
            nc.sync.dma_start(out=wc_sb[:, :], in_=wc[:, :])
            nc.sync.dma_start(out=id12_sb[:, :], in_=ident12[:, :])

            def pro_transpose(g, use_act=False, xg=None):
                """Stage 1: DMA 1024 rows, batched transposes, x2T copies.

                use_act (group 0, cold start): xg was DMA'd up front; put
                half the copies on the then-idle scalar engine to shorten
                the startup chain."""
                if xg is None:
                    xg = xin.tile([128, 8, NF], f32r, tag="xg")
                    src = bass.AP(
                        xr, 1024 * g * NF,
                        [[NF, 128], [128 * NF, 8], [1, NF]],
                    )
                    nc.sync.dma_start(out=xg[:, :, :], in_=src)
                # batched transpose: [128,(4,32)] -> 4 stacked [32,128] blocks
                pt = ps.tile([NF, QL], f32r, tag="ps")
                for s in range(2):
                    for j in range(4):
                        nc.tensor.matmul(
                            pt[:, 512 * s + 128 * j:512 * s + 128 * (j + 1)],
                            xg[:, 4 * s + j, :], idr_sb[:, :],
                            is_transpose=True, start=(j == 0), stop=(j == 3),
                        )
                x2t = x2tp.tile([NF, QL], f32r, tag="x2t")
                for s in range(2):
                    cp = nc.scalar.copy if (use_act and s == 1) \
                        else nc.vector.tensor_copy
                    cp(x2t[:, 512 * s:512 * (s + 1)],
                       pt[:, 512 * s:512 * (s + 1)])
                return x2t

            def pro_pooled(g, x2t, use_act=False):
                """Stage 2: pooledT[:, 1024g:...] = Wc.T @ x2T."""
                pp = ps.tile([DV, QL], f32, tag="ps")
                for s in range(2):
                    nc.tensor.matmul(
                        pp[:, 512 * s:512 * (s + 1)], wc_sb[:, :],
                        x2t[:, 512 * s:512 * (s + 1)],
                        start=True, stop=True,
                    )
                    cp = nc.scalar.copy if (use_act and s == 1) \
                        else nc.vector.tensor_copy
                    cp(pooledT[:, QL * g + 512 * s:QL * g + 512 * (s + 1)],
                       pp[:, 512 * s:512 * (s + 1)])

            def pro_vnat(g, x2t):
                """Stage 3: natural-layout [pooled|1|0] rows -> vn."""
                pn = ps.tile([128, 8 * DV], f32, tag="ps")
                for u in range(8):
                    nc.tensor.matmul(
                        pn[:, DV * u:DV * (u + 1)],
                        x2t[:, 128 * u:128 * (u + 1)], wc_sb[:, :],
                        start=(u == 0), stop=(u == 7),
                    )
                nc.vector.tensor_copy(
                    vn[:, 8 * g:8 * (g + 1), :],
                    pn[:, :].rearrange("p (u d) -> p u d", u=8),
                )

            def emit_s(c):
                st = ps.tile([128, QL], f32, tag="ps")
                lhs = pooledT[0:D, 128 * c:128 * (c + 1)]
                for h in range(2):
                    nc.tensor.matmul(
                        st[:, 512 * h:512 * (h + 1)], lhs,
                        pooledT[0:D, 512 * h:512 * (h + 1)],
                        start=True, stop=True,
                    )
                return st

            def emit_exp_pv(c, st):
                et = epool.tile([128, QL], f32r, tag="E")
                if c == NKV - 1:
                    # halve the last chunk so the epilogue chain (PV -> sv ->
                    # transpose -> divide -> DMA) starts half an exp earlier
                    for h in range(2):
                        nc.scalar.activation(
                            out=et[:, 512 * h:512 * (h + 1)],
                            in_=st[:, 512 * h:512 * (h + 1)], func=EXP)
                        nc.tensor.matmul(
                            pv[:, 512 * h:512 * (h + 1)], vn[:, c, :],
                            et[:, 512 * h:512 * (h + 1)],
                            start=False, stop=True,
                        )
                    return
                nc.scalar.activation(out=et[:, :], in_=st[:, :], func=EXP)
                for h in range(2):
                    nc.tensor.matmul(
                        pv[:, 512 * h:512 * (h + 1)],
                        vn[:, c, :],
                        et[:, 512 * h:512 * (h + 1)],
                        start=(c == 0), stop=(c == NKV - 1),
                    )

            # interleave: prologue(g+1) stages spread across group-g chunk
            # slots so each PE stage has its DVE-copy input ready (no PE
            # queue stalls); exp/PV trail S by 2 chunks. Group 0 runs its
            # whole prologue up front with ACT helping the copies.
            LOOKAHEAD = 2
            s_tiles = {}
            x2t0 = pro_transpose(0, use_act=True, xg=xg0)
            pro_pooled(0, x2t0, use_act=True)
            pro_vnat(0, x2t0)
            x2t_next = None
            for c in range(NKV):
                g_next = c // 8 + 1
                if g_next < NSG:
                    if c % 8 == 0:
                        x2t_next = pro_transpose(g_next)
                    elif c % 8 == 3:
                        pro_pooled(g_next, x2t_next)
                    elif c % 8 == 6:
                        pro_vnat(g_next, x2t_next)
                s_tiles[c] = emit_s(c)
                if c - LOOKAHEAD >= 0:
                    emit_exp_pv(c - LOOKAHEAD, s_tiles.pop(c - LOOKAHEAD))
            for c in range(NKV - LOOKAHEAD, NKV):
                emit_exp_pv(c, s_tiles.pop(c))

            # ---- epilogue: v = acc[0:10]/acc[10], transposed out ----
            # sv is f32r so the strided row-regrouping transposes (partition
            # p <- q rows 8p+j) are legal; the output DMA then writes 80B
            # contiguous per partition instead of 40B granules.
            sv = bigp.tile([DV, QL], f32r, tag="sv")
            for s in range(2):
                nc.scalar.copy(
                    sv[:, 512 * s:512 * (s + 1)], pv[:, 512 * s:512 * (s + 1)])
            vout = bigp.tile([128, 8, D], f32, tag="vout")
            po = ps.tile([128, 8, DV], f32r, tag="ps")
            svap = sv[:, :]
            for j in range(8):
                src8 = bass.AP(svap.tensor, svap.offset + j,
                               [svap.ap[0], [8, 128]])
                nc.tensor.matmul(
                    po[:, j, :], src8, idr_sb[0:DV, 0:DV],
                    is_transpose=True, start=(j == 0), stop=(j == 7),
                )
            rec = outp.tile([128, 8], f32, tag="rec")
            nc.vector.reciprocal(rec[:, :], po[:, :, D])
            rec_b = bass.AP(rec[:, :].tensor, rec[:, :].offset,
                            [rec[:, :].ap[0], [1, 8], [0, D]])
            nc.vector.tensor_mul(vout[:, :, :], po[:, :, 0:D], rec_b)
            dst = bass.AP(v, 0, [[8 * D, 128], [1, 8 * D]])
            nc.sync.dma_start(
                out=dst, in_=vout[:, :, :].rearrange("p j d -> p (j d)"))

    nc.finalize()
    return nc


def _get_nc():
    global _NC
    if _NC is None:
        _NC = _build()
    return _NC


def _host_fold(A, W):
    """Fold adjacency normalization + node pooling into one [32,12] weight.

    Column 10 selects the host-appended ones feature (row 30) so the same
    matmul also produces the augmented-V ones column; row 31/col 11 are
    zero padding (fp32r matmuls need even shapes)."""
    A = np.asarray(A, np.float32)
    W = np.asarray(W, np.float32)
    off = np.ones((3, 3), np.float32) - np.eye(3, dtype=np.float32)
    a = off + A
    d = 0.5 * np.eye(3, dtype=np.float32)
    adj = (d @ a @ d).astype(np.float32)
    c = (0.5 * (adj[0, :] + adj[1, :])).astype(np.float32)
    wcm = np.zeros((NF, DV), np.float32)
    wcm[0:30, 0:D] = np.einsum("n,fo->nfo", c, W).reshape(30, D)
    wcm[30, D] = 1.0
    return wcm


def _host_x2(x):
    x2 = np.zeros((B, NF), np.float32)
    x2[:, 0:30] = np.asarray(x, np.float32).reshape(B, 30)
    x2[:, 30] = 1.0
    return x2


def kernel(x, A, W):
    wcm = _host_fold(A, W)
    x2 = _host_x2(x)
    identr = np.eye(128, dtype=np.float32)
    ident12 = np.eye(DV, dtype=np.float32)

    nc = _get_nc()
    in_maps = [
        {"xr": np.roll(x2, -QL * i, axis=0), "wc": wcm,
         "identr": identr, "ident12": ident12}
        for i in range(NCORES)
    ]
    res = run_bass_kernel_spmd(nc, in_maps, core_ids=list(range(NCORES)))
    return np.concatenate([res.results[i]["v"] for i in range(NCORES)], axis=0)

